# revision 59
# baseline (speedup 1.0000x reference)
"""Mistral-style MHA prefill kernel for Trainium2, 8-way tensor-parallel over heads.

Problem (hardcoded): B=1, S=2048, DIM=4096, 32 q-heads / 8 kv-heads, head_dim=128,
sliding window 2048 (== S, so the mask is exactly causal), rope theta 1e4.

Sharding: core c owns q-heads [4c, 4c+4) and kv-head c. wq/wk/wv are sharded on the
head axis, wo on its input (head) axis; each core computes a full-shape partial
output and the host sums the 8 partials (row-parallel linear + host all-reduce).

Data path is bf16 (matmul stream rate equals f32r at 1 cyc/row, but stationary
loads, DMA and SBUF traffic all halve, and bf16 has no small-N rate penalty so
causal skipping works at 128-column granularity). PSUM accumulation stays f32;
simulated end-to-end error vs the f32 reference is ~4e-3 (gate 2e-2).

Layout strategy (all chosen host-side so the device never transposes activations):
  - x is passed pre-transposed xT [DIM, S]; projections run as W @ x -> [feat, S],
    so Q^T/K^T/V^T [128, S] per head come straight out of PSUM.
  - head_dim is permuted per 32-partition quadrant (16 re rows, then 16 im rows)
    so RoPE pairs sit +-16 apart inside a quadrant: the rotation is a
    stream_shuffle + two muls + one fused sign-multiply-add, all full-width.
    The permutation is score-invariant (applied consistently to Q and K).
  - 1/sqrt(head_dim) is folded into the rope cos/sin tables as sqrt(scale).
  - scores are computed transposed, S_T[k, q] (k on partitions), per 512-wide
    q-block; diagonal k-tiles only compute columns q >= k (128-col granularity),
    masked with one [128,128] upper-triangle multiply.
  - softmax denominator: exp tiles accumulate into a bf16 running sum on DVE
    (4x-rate adds); one ones-matmul per block reduces it across partitions,
    replicated to all 128 PSUM partitions, so reciprocal_approx_fast (DVE
    cost is free-size-based) feeds the normalize-evict directly with no
    broadcast. This keeps the PE free of the 160 denominator streams the
    naive scheme needs.
  - the wo projection is interleaved with attention at fine grain (b-outer,
    head-inner, one wo matmul dripped between attention instructions), so the
    PE never idles on ScalarE exp latency; each block's denominator chain is
    deferred into the next block's fill stream. b=0, which has no wo work to
    hide behind, runs its four heads as one flat round-robin pipeline with
    the deferred sblock-3 V transposes as tail filler.
  - x is DMA'd in [128,1024] host-contiguous pairs; sblock-3 ropes run inside
    phase 2; a burst of dummy matmuls ramps the PE p-state while the first
    weight DMAs are in flight.
  - normalized out^T overwrites the dead qt[h] q-block in place (otn alias).
"""

import numpy as np

B = 1
S = 2048
DIM = 4096
N_HEADS = 32
N_KV = 8
DH = 128
NCORES = 8
HPC = N_HEADS // NCORES  # q heads per core
FQKV = HPC * DH + 2 * DH  # 768 projection rows per core
NKT = S // DH  # 16 k tiles
NQB = S // 512  # 4 q blocks
NDCH = DIM // DH  # 32 contraction chunks

_PROGRAM = None

# stream_shuffle mask: swap 16-partition halves within each 32-partition quadrant
_SWAP16 = [(i + 16) % 32 for i in range(32)]


def _head_perm():
    """Permutation of head_dim rows: quadrant q holds [re_16q..re_16q+15,
    im_16q..im_16q+15], so RoPE pairs are +-16 apart within a quadrant."""
    p = np.empty(DH, dtype=np.int64)
    for row in range(DH):
        q, j = divmod(row, 32)
        i = 16 * q + (j % 16)  # rope pair index
        p[row] = 2 * i + (0 if j < 16 else 1)
    return p


def _build_program():
    import concourse.bacc as bacc
    import concourse.mybir as mybir
    import concourse.tile as tile

    F32 = mybir.dt.float32
    F32R = mybir.dt.float32r
    BF16 = mybir.dt.bfloat16
    EXP = mybir.ActivationFunctionType.Exp

    nc = bacc.Bacc("TRN2", target_bir_lowering=False, debug=False,
                   enable_asserts=False)

    # x pre-paired host-side: [128, pair, sb, 1024] flattened so each
    # [128, 1024] DMA (two d-chunks of one s-block) is fully contiguous
    xT2 = nc.dram_tensor("xT2", [DH, (NDCH // 2) * NQB * 1024], BF16,
                         kind="ExternalInput")
    wqkvT = nc.dram_tensor("wqkvT", [DIM, FQKV], BF16, kind="ExternalInput")
    woT = nc.dram_tensor("woT", [HPC * DH, DIM], BF16, kind="ExternalInput")
    csA_d = nc.dram_tensor("csA", [DH, S], F32, kind="ExternalInput")
    csB_d = nc.dram_tensor("csB", [DH, S], F32, kind="ExternalInput")
    sign_d = nc.dram_tensor("sign", [DH, 1], F32, kind="ExternalInput")
    tri_d = nc.dram_tensor("tri", [DH, DH], BF16, kind="ExternalInput")
    identr_d = nc.dram_tensor("identr", [DH, DH], F32R, kind="ExternalInput")
    ones128_d = nc.dram_tensor("ones128", [DH, DH], BF16, kind="ExternalInput")
    out_d = nc.dram_tensor("out", [S, DIM], F32, kind="ExternalOutput")

    with tile.TileContext(nc) as tc:
        with (
            tc.tile_pool(name="consts", bufs=1) as cpool,
            tc.tile_pool(name="persist", bufs=1) as ppool,
        ):
            csA_sb = cpool.tile([DH, S], F32)
            csB_sb = cpool.tile([DH, S], F32)
            sign_sb = cpool.tile([DH, 1], F32)
            tri_sb = cpool.tile([DH, DH], BF16)
            identr_sb = cpool.tile([DH, DH], F32R)
            ones128_sb = cpool.tile([DH, DH], BF16)

            qt = [ppool.tile([DH, S], BF16, name=f"qt{h}") for h in range(HPC)]
            kt = ppool.tile([DH, S], BF16)
            vn = ppool.tile([DH, S], BF16)  # V in normal layout
            wo_sb = ppool.tile([DH, HPC * DIM], BF16)
            otn = qt  # attention block b is the last reader of its qt columns

            # rope/raw/vt pools span both phases: sblock 3's ropes and V
            # transposes are deferred into phase 2 (their outputs are only
            # read by later blocks)
            rope_ctx = tc.tile_pool(name="ropet", bufs=4)
            rtp = rope_ctx.__enter__()
            raw_ctx = tc.tile_pool(name="rawsb", bufs=10)
            rawpool = raw_ctx.__enter__()
            vt_ctx = tc.tile_pool(name="vtt", bufs=2)
            vtp = vt_ctx.__enter__()
            a0e_ctx = tc.tile_pool(name="att0e", bufs=16)
            a0epool = a0e_ctx.__enter__()
            a0a_ctx = tc.tile_pool(name="att0acc", bufs=4)
            a0apool = a0a_ctx.__enter__()

            # ---------------- Phase 1: QKV projection + RoPE ----------------
            with (
                tc.tile_pool(name="xin", bufs=10) as xpool,
                tc.tile_pool(name="qkvps", bufs=6, space="PSUM") as qps,
                tc.tile_pool(name="trps", bufs=2, space="PSUM") as trps,
                tc.tile_pool(name="wsb", bufs=1) as wpool,
            ):
                w_sb = wpool.tile([DH, NDCH * FQKV], BF16)

                def emit_sblock(sb_i, mid=None, pre=None):
                    ps = [qps.tile([DH, 512], F32, name=f"ps{f}", tag="ps")
                          for f in range(6)]
                    for dp in range(NDCH // 2):
                        if pre is not None:
                            pre(dp)
                        if dp == 8 and mid is not None:
                            # previous sblock's ropes drain on DVE here, in
                            # the middle of this sblock, so the DVE queue is
                            # clear for the PSUM evictions at sblock end
                            mid()
                        xt = xpool.tile([DH, 1024], BF16, name="xt", tag="xt")
                        xeng = nc.sync if dp % 2 == 0 else nc.gpsimd
                        xbase = (dp * NQB + sb_i) * 1024
                        xeng.dma_start(xt[:], xT2[:, xbase:xbase + 1024])
                        if sb_i == 0 and dp == 2:
                            # tiny consts needed before sblock 0 ends (identr
                            # by its V transposes)
                            nc.gpsimd.dma_start(identr_sb[:], identr_d[:])
                            nc.gpsimd.dma_start(sign_sb[:], sign_d[:])
                        if sb_i == 0 and dp == 13:
                            # big rope tables ride the ACT ring behind most of
                            # the qkv weights, clear of the x streams; must be
                            # EMITTED before the sblock-0 ropes that read them
                            nc.scalar.dma_start(csA_sb[:], csA_d[:])
                            nc.scalar.dma_start(csB_sb[:], csB_d[:])
                        if sb_i == 0 and dp == 15:
                            nc.scalar.dma_start(tri_sb[:], tri_d[:])
                            nc.scalar.dma_start(ones128_sb[:], ones128_d[:])
                        if sb_i == 1 and dp in (4, 7, 10, 13):
                            # wo weights stream on the ACT ring once the qkv
                            # weights are in; needed first by WO(b=0)
                            ch = {4: 0, 7: 1, 10: 2, 13: 3}[dp]
                            nc.scalar.dma_start(
                                wo_sb[:, ch * DIM:(ch + 1) * DIM],
                                woT[ch * DH:(ch + 1) * DH, :])
                        for c in range(2):
                            d = 2 * dp + c
                            if sb_i == 0:
                                # weight loads ride the ACT HWDGE ring,
                                # concurrent with the x loads
                                nc.scalar.dma_start(
                                    w_sb[:, d * FQKV:(d + 1) * FQKV],
                                    wqkvT[d * DH:(d + 1) * DH, :])
                            for f in range(6):
                                nc.tensor.matmul(
                                    ps[f][:],
                                    w_sb[:, d * FQKV + f * DH:
                                         d * FQKV + (f + 1) * DH],
                                    xt[:, c * 512:(c + 1) * 512],
                                    start=(d == 0), stop=(d == NDCH - 1))
                    # raw PSUM->SBUF evictions (alternating ACT/DVE) free the
                    # accumulator banks quickly; RoPE runs later from SBUF.
                    vt_t = vtp.tile([DH, 512], F32R, name="vt_t", tag="vt")
                    nc.scalar.copy(vt_t[:], ps[5][:])
                    raws = {}
                    for i, f in enumerate([0, 4, 2, 1, 3]):
                        raw = rawpool.tile([DH, 512], F32R, name="raw", tag="raw")
                        raws[f] = raw
                        if i % 2 == 1:
                            nc.scalar.copy(raw[:], ps[f][:])
                        else:
                            nc.vector.tensor_copy(raw[:], ps[f][:])
                    if sb_i < NQB - 1:
                        for t in range(4):
                            tp = trps.tile([DH, DH], F32R, name="tp", tag="tp")
                            nc.tensor.transpose(
                                tp[:], vt_t[:, t * DH:(t + 1) * DH],
                                identr_sb[:])
                            j = sb_i * 4 + t
                            nc.vector.tensor_copy(vn[:, j * DH:(j + 1) * DH],
                                                  tp[:])
                    return raws, vt_t

                def emit_rope(f, sb_i, raw):
                    # head_dim permuted so pairs sit +-16 apart within each
                    # 32-partition quadrant: dest = p1 + sign*p3 where
                    # p1 = q*cos, p3 = halfswap(q)*sin. dest is bf16.
                    col = slice(sb_i * 512, (sb_i + 1) * 512)
                    dest = qt[f] if f < HPC else kt
                    qs_t = rtp.tile([DH, 512], F32, name="qs_t", tag="qs")
                    p1 = rtp.tile([DH, 512], F32, name="p1", tag="p1")
                    nc.vector.stream_shuffle(qs_t[:], raw[:], _SWAP16)
                    nc.vector.tensor_mul(p1[:], raw[:], csA_sb[:, col])
                    nc.vector.tensor_mul(qs_t[:], qs_t[:], csB_sb[:, col])
                    nc.vector.scalar_tensor_tensor(
                        dest[:, col], qs_t[:], sign_sb[:], p1[:],
                        mybir.AluOpType.mult, mybir.AluOpType.add)

                # PE p-state warmup: ~60 dummy matmuls on garbage SBUF
                # (output never read) ramp the clock to max while the first
                # weight/x DMAs are still in flight
                wz = rawpool.tile([DH, 512], F32, name="wz", tag="raw")
                nc.vector.memset(wz[:, :DH], 0.0)
                for wi in range(15):
                    wps = trps.tile([DH, DH], F32, name="wps", tag="tp")
                    nc.tensor.matmul(wps[:], wz[:, :DH], wz[:, :DH],
                                     start=True, stop=True)

                # ATT0's scores/exp/mask/denominator-accumulate run inside
                # sblock 3's stream (one tiny matmul per d-pair, exp latency
                # fully hidden by the QKV matmuls between), so the ATT0 phase
                # reduces to its PV streams and tails
                a0_es = {}
                a0_accs = []

                def att0_pre(dp):
                    k, h = divmod(dp, HPC)
                    s0 = k * DH
                    e = a0epool.tile([DH, 512], BF16, name="a0e", tag="ae")
                    a0_es[(h, k)] = e
                    sp = trps.tile([DH, 512], F32, name="a0sp", tag="tp")
                    nc.tensor.matmul(sp[:, s0:], kt[:, k * DH:(k + 1) * DH],
                                     qt[h][:, s0:512], start=True, stop=True)
                    nc.scalar.activation(e[:, s0:], sp[:, s0:], EXP)
                    nc.vector.tensor_mul(e[:, s0:s0 + DH], e[:, s0:s0 + DH],
                                         tri_sb[:])
                    if k == 0:
                        acc = a0apool.tile([DH, 512], BF16, name="a0acc",
                                           tag="aa")
                        a0_accs.append(acc)
                        nc.vector.tensor_copy(acc[:], e[:])
                    else:
                        nc.vector.tensor_add(a0_accs[h][:, s0:],
                                             a0_accs[h][:, s0:], e[:, s0:])

                raws3 = vt3 = None
                pending = None
                for sb_i in range(NQB):
                    raws, vt_t = emit_sblock(
                        sb_i, mid=pending,
                        pre=att0_pre if sb_i == NQB - 1 else None)
                    pending = None
                    if sb_i == NQB - 1:
                        # sblock 3's ropes and V transposes are deferred into
                        # phase 2 so their chains don't gate the first scores
                        raws3, vt3 = raws, vt_t
                        continue

                    def pending(raws=raws, sb_i=sb_i):
                        # k first, then q head 0, then the rest
                        for f in [4, 0, 1, 2, 3]:
                            emit_rope(f, sb_i, raws[f])
                if pending is not None:
                    pending()

            # -------- Phase 2: attention (b-outer) interleaved with WO -------
            with (
                tc.tile_pool(name="mps", bufs=8, space="PSUM") as mps,
                tc.tile_pool(name="esb", bufs=10) as epool,
                tc.tile_pool(name="eacc", bufs=4) as eaccp,
                tc.tile_pool(name="bcb", bufs=4) as bcbp,
                tc.tile_pool(name="evsb", bufs=6) as evpool,
            ):
                def wo_units(b, dve_only=True):
                    """The wo projection for q-block b as a list of emission
                    thunks (one PE matmul or one eviction step each), to be
                    drip-fed between attention matmuls of block b+1 so the PE
                    queue never drains on exp latency."""
                    units = []
                    for sti in range(4):
                        st = 4 * b + sti
                        scol = slice(st * DH, (st + 1) * DH)
                        for dh_i in range(2):
                            for jp in range(2):
                                pair = []  # bound late via default args

                                def alloc(pair=pair):
                                    pair.append(mps.tile(
                                        [DH, 512], F32, name="pw", tag="ps"))

                                def mm(jj, h, pair=pair, scol=scol, dh_i=dh_i,
                                       jp=jp):
                                    off = dh_i * 2048 + (jp * 2 + jj) * 512
                                    nc.tensor.matmul(
                                        pair[jj][:], otn[h][:, scol],
                                        wo_sb[:, h * DIM + off:
                                              h * DIM + off + 512],
                                        start=(h == 0), stop=(h == HPC - 1))

                                def ev_emit(pair=pair, scol=scol, dh_i=dh_i,
                                            jp=jp, sti=sti, st=st, b=b,
                                            dve_only=dve_only):
                                    ev = evpool.tile([DH, 1024], F32,
                                                     name="ev", tag="ev")
                                    if dve_only:
                                        # ACT is exp-saturated in the batches
                                        # these units interleave into
                                        nc.vector.tensor_copy(ev[:, 0:512],
                                                              pair[0][:])
                                        nc.vector.tensor_copy(ev[:, 512:1024],
                                                              pair[1][:])
                                    elif (sti + jp) % 2 == 0:
                                        nc.scalar.copy(ev[:, 0:512], pair[0][:])
                                        nc.vector.tensor_copy(ev[:, 512:1024],
                                                              pair[1][:])
                                    else:
                                        nc.vector.tensor_copy(ev[:, 0:512],
                                                              pair[0][:])
                                        nc.scalar.copy(ev[:, 512:1024],
                                                       pair[1][:])
                                    base = dh_i * 2048 + jp * 1024
                                    if b == NQB - 1:
                                        # last batch: halve each out write
                                        # across both rings so the final
                                        # transfer isn't exposed in the tail
                                        nc.sync.dma_start(
                                            out_d[scol, base:base + 512],
                                            ev[:, 0:512])
                                        nc.scalar.dma_start(
                                            out_d[scol, base + 512:base + 1024],
                                            ev[:, 512:1024])
                                    elif (st + dh_i + jp) % 2 == 0:
                                        nc.sync.dma_start(
                                            out_d[scol, base:base + 1024], ev[:])
                                    else:
                                        nc.scalar.dma_start(
                                            out_d[scol, base:base + 1024], ev[:])

                                for jj in range(2):
                                    for h in range(HPC):
                                        if h == 0:
                                            units.append(
                                                (lambda jj=jj, a=alloc, m=mm,
                                                 h=h: (a(), m(jj, h))))
                                        else:
                                            units.append(
                                                (lambda jj=jj, m=mm, h=h:
                                                 m(jj, h)))
                                units.append(ev_emit)
                    return units

                def emit_block(h, b, filler=None):
                    cb = slice(b * 512, (b + 1) * 512)
                    nk = 4 * b + 4  # k tiles contributing to this q block
                    ot_b = mps.tile([DH, 512], F32, name="ot", tag="ps")
                    eacc = eaccp.tile([DH, 512], BF16, name="eacc", tag="ea")
                    e_tiles = [None] * nk

                    def fill():
                        if filler is not None:
                            filler()

                    def s0_of(k):
                        return (k - 4 * b) * DH if k >= 4 * b else 0

                    def emit_scores(k):
                        s0 = s0_of(k)
                        e = epool.tile([DH, 512], BF16, name="E", tag="E")
                        e_tiles[k] = e
                        sp = mps.tile([DH, 512], F32, name="sp", tag="ps")
                        nc.tensor.matmul(
                            sp[:, s0:], kt[:, k * DH:(k + 1) * DH],
                            qt[h][:, b * 512 + s0:(b + 1) * 512],
                            start=True, stop=True)
                        nc.scalar.activation(e[:, s0:], sp[:, s0:], EXP)
                        if k >= 4 * b:
                            # diagonal tile: upper-triangle mask on its strip
                            nc.vector.tensor_mul(
                                e[:, s0:s0 + DH], e[:, s0:s0 + DH], tri_sb[:])
                        if k == 0:
                            nc.vector.tensor_copy(eacc[:], e[:])
                        else:
                            nc.vector.tensor_add(eacc[:, s0:], eacc[:, s0:],
                                                 e[:, s0:])

                    def emit_pv(k):
                        s0 = s0_of(k)
                        nc.tensor.matmul(
                            ot_b[:, s0:], vn[:, k * DH:(k + 1) * DH],
                            e_tiles[k][:, s0:], start=(k == 0),
                            stop=(k == nk - 1), skip_group_check=True)

                    # 3-deep software pipeline: scores run three steps ahead of
                    # PV; wo-projection fill matmuls slot in between
                    depth = min(4, nk)
                    for k in range(depth):
                        emit_scores(k)
                        fill()
                    for k in range(depth, nk):
                        emit_scores(k)
                        fill()
                        emit_pv(k - depth)
                        fill()
                    for k in range(nk - depth, nk):
                        emit_pv(k)
                        fill()

                    # denominator chain, deferred: one ones-matmul reduces the
                    # f32 running sum across partitions (already replicated to
                    # all 128 PSUM partitions, so no broadcast is needed),
                    # recip, then normalize-evict. Returned as a thunk so it
                    # runs a few instructions into the NEXT block, after the
                    # eacc adds have drained.
                    def finish():
                        dn_b = mps.tile([DH, 512], F32, name="dn", tag="ps")
                        nc.tensor.matmul(dn_b[:], ones128_sb[:], eacc[:],
                                         start=True, stop=True)
                        bc = bcbp.tile([DH, 512], F32, name="bc", tag="bc")
                        nc.vector.reciprocal_approx_fast(bc[:], dn_b[:])
                        nc.vector.tensor_mul(otn[h][:, cb], ot_b[:], bc[:])

                    return finish

                def run_batch(b, units):
                    """Emit attention for q-block b, drip-feeding `units`
                    (wo work for block b-1) between its PE instructions."""
                    pend = list(units)
                    state = {"i": 0, "slots": HPC * 2 * (4 * b + 4)}

                    def filler():
                        rem = len(pend) - state["i"]
                        if state["slots"] > 0:
                            n = min(rem, -(-rem // state["slots"]))
                            state["slots"] -= 1
                        else:
                            n = rem
                        for _ in range(n):
                            pend[state["i"]]()
                            state["i"] += 1

                    for h in range(HPC):
                        fin = emit_block(h, b, filler)
                        # the block's denominator/normalize chain drains at
                        # the head of the next block's fill stream, once the
                        # eacc adds have had time to complete
                        pend.insert(min(state["i"] + 2, len(pend)), fin)
                        if b == NQB - 1 and h < HPC - 1:
                            # deferred sblock-3 rope for the next head's
                            # q-block; runs on DVE behind this block's tail
                            emit_rope(h + 1, 3, raws3[h + 1])
                    while state["i"] < len(pend):
                        pend[state["i"]]()
                        state["i"] += 1

                def emit_att0_flat():
                    """b=0's scores/exp ran inside sblock 3; only the PV
                    streams and denominator tails remain here."""
                    nk = 4
                    ots = []
                    for h in range(HPC):
                        ots.append(mps.tile([DH, 512], F32, name=f"ot{h}",
                                            tag="ps"))
                    eaccs = a0_accs

                    for k in range(nk):
                        for h in range(HPC):
                            s0 = k * DH
                            nc.tensor.matmul(ots[h][:, s0:],
                                             vn[:, k * DH:(k + 1) * DH],
                                             a0_es[(h, k)][:, s0:],
                                             start=(k == 0),
                                             stop=(k == nk - 1),
                                             skip_group_check=True)
                    # tail: denominator chains interleaved with the deferred
                    # sblock-3 V transposes (PE filler while eacc adds drain)
                    for h in range(HPC):
                        tp = mps.tile([DH, DH], F32R, name="tp", tag="ps")
                        nc.tensor.transpose(tp[:], vt3[:, h * DH:(h + 1) * DH],
                                            identr_sb[:])
                        nc.vector.tensor_copy(
                            vn[:, (12 + h) * DH:(13 + h) * DH], tp[:])
                        dn = mps.tile([DH, 512], F32, name="dn", tag="ps")
                        nc.tensor.matmul(dn[:], ones128_sb[:], eaccs[h][:],
                                         start=True, stop=True)
                        bc = bcbp.tile([DH, 512], F32, name="bc", tag="bc")
                        nc.vector.reciprocal_approx_fast(bc[:], dn[:])
                        nc.vector.tensor_mul(otn[h][:, 0:512], ots[h][:],
                                             bc[:])

                emit_att0_flat()
                # kt/qt[0] sblock-3 ropes run on DVE during ATT(1)/ATT(2)
                emit_rope(0, 3, raws3[0])
                run_batch(1, wo_units(0))
                emit_rope(4, 3, raws3[4])
                run_batch(2, wo_units(1))
                run_batch(3, wo_units(2))
                for u in wo_units(3, dve_only=False):
                    u()

            a0a_ctx.__exit__(None, None, None)
            a0e_ctx.__exit__(None, None, None)
            vt_ctx.__exit__(None, None, None)
            raw_ctx.__exit__(None, None, None)
            rope_ctx.__exit__(None, None, None)

    nc.compile()
    return nc


def get_program():
    global _PROGRAM
    if _PROGRAM is None:
        _PROGRAM = _build_program()
    return _PROGRAM


def make_in_maps(inputs):
    """Host-side sharding / layout prep. Returns one input dict per core."""
    import ml_dtypes
    bf16 = ml_dtypes.bfloat16

    x = np.asarray(inputs["x"], dtype=np.float32)
    wq = np.asarray(inputs["wq"], dtype=np.float32)
    wk = np.asarray(inputs["wk"], dtype=np.float32)
    wv = np.asarray(inputs["wv"], dtype=np.float32)
    wo = np.asarray(inputs["wo"], dtype=np.float32)
    cos = np.asarray(inputs["freqs_cos"], dtype=np.float32)  # (S, 64)
    sin = np.asarray(inputs["freqs_sin"], dtype=np.float32)

    # x transposed then re-laid-out so each [128, 1024] tile (two d-chunks of
    # one 512-wide s-block) is contiguous: [p, pair, sb, c*512+q]
    xT = x.reshape(S, DIM).T  # (DIM, S)
    xT2 = np.ascontiguousarray(
        xT.reshape(NDCH // 2, 2, DH, NQB, 512)
        .transpose(2, 0, 3, 1, 4)
        .reshape(DH, (NDCH // 2) * NQB * 1024)).astype(bf16)

    perm = _head_perm()
    sq = np.float32(DH ** -0.25)  # sqrt of 1/sqrt(head_dim), folded into Q and K
    rows = np.arange(DH)
    pair_idx = 16 * (rows // 32) + (rows % 32) % 16
    csA = np.ascontiguousarray(cos.T[pair_idx] * sq)          # (128, S)
    csB = np.ascontiguousarray(sin.T[pair_idx] * sq)
    sign = np.where((rows % 32) < 16, -1.0, 1.0).astype(np.float32).reshape(DH, 1)
    tri = np.triu(np.ones((DH, DH), dtype=np.float32)).astype(bf16)
    identr = np.eye(DH, dtype=np.float32)
    ones128 = np.ones((DH, DH), dtype=np.float32).astype(bf16)

    wqh = wq.reshape(N_HEADS, DH, DIM)[:, perm, :]
    wkh = wk.reshape(N_KV, DH, DIM)[:, perm, :]
    wvh = wv.reshape(N_KV, DH, DIM)

    in_maps = []
    for c in range(NCORES):
        w_c = np.concatenate(
            [wqh[HPC * c:HPC * (c + 1)].reshape(HPC * DH, DIM),
             wkh[c], wvh[c]], 0)  # (768, DIM)
        wqkvT = np.ascontiguousarray(w_c.T).astype(bf16)  # (DIM, 768)
        woT = np.ascontiguousarray(
            wo[:, HPC * DH * c:HPC * DH * (c + 1)].T).astype(bf16)
        in_maps.append({
            "xT2": xT2, "wqkvT": wqkvT, "woT": woT,
            "csA": csA, "csB": csB, "sign": sign, "tri": tri,
            "identr": identr, "ones128": ones128,
        })
    return in_maps


def _ensure_ntff_hook():
    """The agent image's `antenv` lacks `axon_hooks`; recreate it so
    run_bass_kernel_spmd(trace=True) can capture NTFF profiles. Mirrors
    trn_agent_boot/trn_boot.py::_ntff_profile_via_ctypes."""
    import sys
    try:
        from antenv.axon_hooks import get_axon_ntff_profile_hook  # noqa: F401
        return
    except ImportError:
        pass
    import contextlib
    import ctypes
    import types

    so_path = "/opt/axon/libaxon_pjrt.so"
    hook = None
    try:
        lib = ctypes.CDLL(so_path)
        if hasattr(lib, "axon_start_nrt_profile"):
            lib.axon_start_nrt_profile.argtypes = [
                ctypes.POINTER(ctypes.c_int64), ctypes.c_size_t]
            lib.axon_start_nrt_profile.restype = ctypes.c_int64
            lib.axon_stop_nrt_profile.argtypes = [ctypes.c_char_p]
            lib.axon_stop_nrt_profile.restype = ctypes.c_int64

            @contextlib.contextmanager
            def _hook(output_dir, device_ids):
                import jax
                jax.devices()
                if device_ids:
                    ids = (ctypes.c_int64 * len(device_ids))(*device_ids)
                    rc = lib.axon_start_nrt_profile(ids, len(device_ids))
                else:
                    rc = lib.axon_start_nrt_profile(None, 0)
                if rc != 0:
                    raise RuntimeError(f"axon_start_nrt_profile rc={rc}")
                try:
                    yield
                finally:
                    n = lib.axon_stop_nrt_profile(str(output_dir).encode())
                    print(f"profile: {n} file(s) written to {output_dir}")

            hook = _hook
    except OSError:
        pass

    mod = types.ModuleType("antenv.axon_hooks")
    mod._hook = hook
    mod.get_axon_ntff_profile_hook = lambda: mod._hook
    mod.set_axon_ntff_profile_hook = lambda h: setattr(mod, "_hook", h)
    sys.modules["antenv.axon_hooks"] = mod


def run(inputs, trace=False):
    from concourse.bass_utils import run_bass_kernel_spmd
    if trace:
        _ensure_ntff_hook()
    nc = get_program()
    in_maps = make_in_maps(inputs)
    res = run_bass_kernel_spmd(nc, in_maps, core_ids=list(range(NCORES)),
                               trace=trace)
    acc = np.zeros((S, DIM), dtype=np.float32)
    for r in res.results:
        acc += np.asarray(r["out"], dtype=np.float32)
    return acc.reshape(B, S, DIM), res


def kernel(**inputs):
    out, _ = run(inputs, trace=False)
    return out


# revision 60
# speedup vs baseline: 1.0043x; 1.0043x over previous
"""Mistral-style MHA prefill kernel for Trainium2, 8-way tensor-parallel over heads.

Problem (hardcoded): B=1, S=2048, DIM=4096, 32 q-heads / 8 kv-heads, head_dim=128,
sliding window 2048 (== S, so the mask is exactly causal), rope theta 1e4.

Sharding: core c owns q-heads [4c, 4c+4) and kv-head c. wq/wk/wv are sharded on the
head axis, wo on its input (head) axis; each core computes a full-shape partial
output and the host sums the 8 partials (row-parallel linear + host all-reduce).

Data path is bf16 (matmul stream rate equals f32r at 1 cyc/row, but stationary
loads, DMA and SBUF traffic all halve, and bf16 has no small-N rate penalty so
causal skipping works at 128-column granularity). PSUM accumulation stays f32;
simulated end-to-end error vs the f32 reference is ~4e-3 (gate 2e-2).

Layout strategy (all chosen host-side so the device never transposes activations):
  - x is passed pre-transposed xT [DIM, S]; projections run as W @ x -> [feat, S],
    so Q^T/K^T/V^T [128, S] per head come straight out of PSUM.
  - head_dim is permuted per 32-partition quadrant (16 re rows, then 16 im rows)
    so RoPE pairs sit +-16 apart inside a quadrant: the rotation is a
    stream_shuffle + two muls + one fused sign-multiply-add, all full-width.
    The permutation is score-invariant (applied consistently to Q and K).
  - 1/sqrt(head_dim) is folded into the rope cos/sin tables as sqrt(scale).
  - scores are computed transposed, S_T[k, q] (k on partitions), per 512-wide
    q-block; diagonal k-tiles only compute columns q >= k (128-col granularity),
    masked with one [128,128] upper-triangle multiply.
  - softmax denominator: exp tiles accumulate into a bf16 running sum on DVE
    (4x-rate adds); one ones-matmul per block reduces it across partitions,
    replicated to all 128 PSUM partitions, so reciprocal_approx_fast (DVE
    cost is free-size-based) feeds the normalize-evict directly with no
    broadcast. This keeps the PE free of the 160 denominator streams the
    naive scheme needs.
  - the wo projection is interleaved with attention at fine grain (b-outer,
    head-inner, one wo matmul dripped between attention instructions), so the
    PE never idles on ScalarE exp latency; each block's denominator chain is
    deferred into the next block's fill stream. b=0, which has no wo work to
    hide behind, runs its four heads as one flat round-robin pipeline with
    the deferred sblock-3 V transposes as tail filler.
  - x is DMA'd in [128,1024] host-contiguous pairs; sblock-3 ropes run inside
    phase 2; a burst of dummy matmuls ramps the PE p-state while the first
    weight DMAs are in flight.
  - normalized out^T overwrites the dead qt[h] q-block in place (otn alias).
"""

import numpy as np

B = 1
S = 2048
DIM = 4096
N_HEADS = 32
N_KV = 8
DH = 128
NCORES = 8
HPC = N_HEADS // NCORES  # q heads per core
FQKV = HPC * DH + 2 * DH  # 768 projection rows per core
NKT = S // DH  # 16 k tiles
NQB = S // 512  # 4 q blocks
NDCH = DIM // DH  # 32 contraction chunks

_PROGRAM = None

# stream_shuffle mask: swap 16-partition halves within each 32-partition quadrant
_SWAP16 = [(i + 16) % 32 for i in range(32)]


def _head_perm():
    """Permutation of head_dim rows: quadrant q holds [re_16q..re_16q+15,
    im_16q..im_16q+15], so RoPE pairs are +-16 apart within a quadrant."""
    p = np.empty(DH, dtype=np.int64)
    for row in range(DH):
        q, j = divmod(row, 32)
        i = 16 * q + (j % 16)  # rope pair index
        p[row] = 2 * i + (0 if j < 16 else 1)
    return p


def _build_program():
    import concourse.bacc as bacc
    import concourse.mybir as mybir
    import concourse.tile as tile

    F32 = mybir.dt.float32
    F32R = mybir.dt.float32r
    BF16 = mybir.dt.bfloat16
    EXP = mybir.ActivationFunctionType.Exp

    nc = bacc.Bacc("TRN2", target_bir_lowering=False, debug=False,
                   enable_asserts=False)

    # x pre-paired host-side: [128, pair, sb, 1024] flattened so each
    # [128, 1024] DMA (two d-chunks of one s-block) is fully contiguous
    xT2 = nc.dram_tensor("xT2", [DH, (NDCH // 2) * NQB * 1024], BF16,
                         kind="ExternalInput")
    wqkvT = nc.dram_tensor("wqkvT", [DIM, FQKV], BF16, kind="ExternalInput")
    woT = nc.dram_tensor("woT", [HPC * DH, DIM], BF16, kind="ExternalInput")
    csA_d = nc.dram_tensor("csA", [DH, S], F32, kind="ExternalInput")
    csB_d = nc.dram_tensor("csB", [DH, S], F32, kind="ExternalInput")
    sign_d = nc.dram_tensor("sign", [DH, 1], F32, kind="ExternalInput")
    tri_d = nc.dram_tensor("tri", [DH, DH], BF16, kind="ExternalInput")
    identr_d = nc.dram_tensor("identr", [DH, DH], F32R, kind="ExternalInput")
    ones128_d = nc.dram_tensor("ones128", [DH, DH], BF16, kind="ExternalInput")
    out_d = nc.dram_tensor("out", [S, DIM], F32, kind="ExternalOutput")

    with tile.TileContext(nc) as tc:
        with (
            tc.tile_pool(name="consts", bufs=1) as cpool,
            tc.tile_pool(name="persist", bufs=1) as ppool,
        ):
            csA_sb = cpool.tile([DH, S], F32)
            csB_sb = cpool.tile([DH, S], F32)
            sign_sb = cpool.tile([DH, 1], F32)
            tri_sb = cpool.tile([DH, DH], BF16)
            identr_sb = cpool.tile([DH, DH], F32R)
            ones128_sb = cpool.tile([DH, DH], BF16)

            qt = [ppool.tile([DH, S], BF16, name=f"qt{h}") for h in range(HPC)]
            kt = ppool.tile([DH, S], BF16)
            vn = ppool.tile([DH, S], BF16)  # V in normal layout
            wo_sb = ppool.tile([DH, HPC * DIM], BF16)
            otn = qt  # attention block b is the last reader of its qt columns

            # rope/raw/vt pools span both phases: sblock 3's ropes and V
            # transposes are deferred into phase 2 (their outputs are only
            # read by later blocks)
            rope_ctx = tc.tile_pool(name="ropet", bufs=4)
            rtp = rope_ctx.__enter__()
            raw_ctx = tc.tile_pool(name="rawsb", bufs=10)
            rawpool = raw_ctx.__enter__()
            vt_ctx = tc.tile_pool(name="vtt", bufs=2)
            vtp = vt_ctx.__enter__()
            a0e_ctx = tc.tile_pool(name="att0e", bufs=16)
            a0epool = a0e_ctx.__enter__()
            a0a_ctx = tc.tile_pool(name="att0acc", bufs=4)
            a0apool = a0a_ctx.__enter__()

            # ---------------- Phase 1: QKV projection + RoPE ----------------
            with (
                tc.tile_pool(name="xin", bufs=10) as xpool,
                tc.tile_pool(name="qkvps", bufs=6, space="PSUM") as qps,
                tc.tile_pool(name="trps", bufs=2, space="PSUM") as trps,
                tc.tile_pool(name="wsb", bufs=1) as wpool,
            ):
                w_sb = wpool.tile([DH, NDCH * FQKV], BF16)

                def emit_sblock(sb_i, mid=None, pre=None):
                    ps = [qps.tile([DH, 512], F32, name=f"ps{f}", tag="ps")
                          for f in range(6)]
                    for dp in range(NDCH // 2):
                        if pre is not None:
                            pre(dp)
                        if dp == 8 and mid is not None:
                            # previous sblock's ropes drain on DVE here, in
                            # the middle of this sblock, so the DVE queue is
                            # clear for the PSUM evictions at sblock end
                            mid()
                        xt = xpool.tile([DH, 1024], BF16, name="xt", tag="xt")
                        xeng = nc.sync if dp % 2 == 0 else nc.gpsimd
                        xbase = (dp * NQB + sb_i) * 1024
                        xeng.dma_start(xt[:], xT2[:, xbase:xbase + 1024])
                        if sb_i == 0 and dp == 2:
                            # tiny consts needed before sblock 0 ends (identr
                            # by its V transposes)
                            nc.gpsimd.dma_start(identr_sb[:], identr_d[:])
                            nc.gpsimd.dma_start(sign_sb[:], sign_d[:])
                        if sb_i == 0 and dp == 13:
                            # big rope tables ride the ACT ring behind most of
                            # the qkv weights, clear of the x streams; must be
                            # EMITTED before the sblock-0 ropes that read them
                            nc.scalar.dma_start(csA_sb[:], csA_d[:])
                            nc.scalar.dma_start(csB_sb[:], csB_d[:])
                        if sb_i == 0 and dp == 15:
                            nc.scalar.dma_start(tri_sb[:], tri_d[:])
                            nc.scalar.dma_start(ones128_sb[:], ones128_d[:])
                        if sb_i == 1 and dp in (4, 7, 10, 13):
                            # wo weights stream on the ACT ring once the qkv
                            # weights are in; needed first by WO(b=0)
                            ch = {4: 0, 7: 1, 10: 2, 13: 3}[dp]
                            nc.scalar.dma_start(
                                wo_sb[:, ch * DIM:(ch + 1) * DIM],
                                woT[ch * DH:(ch + 1) * DH, :])
                        for c in range(2):
                            d = 2 * dp + c
                            if sb_i == 0:
                                # weight loads ride the ACT HWDGE ring,
                                # concurrent with the x loads
                                nc.scalar.dma_start(
                                    w_sb[:, d * FQKV:(d + 1) * FQKV],
                                    wqkvT[d * DH:(d + 1) * DH, :])
                            for f in range(6):
                                nc.tensor.matmul(
                                    ps[f][:],
                                    w_sb[:, d * FQKV + f * DH:
                                         d * FQKV + (f + 1) * DH],
                                    xt[:, c * 512:(c + 1) * 512],
                                    start=(d == 0), stop=(d == NDCH - 1))
                    # raw PSUM->SBUF evictions (alternating ACT/DVE) free the
                    # accumulator banks quickly; RoPE runs later from SBUF.
                    vt_t = vtp.tile([DH, 512], F32R, name="vt_t", tag="vt")
                    nc.scalar.copy(vt_t[:], ps[5][:])
                    raws = {}
                    for i, f in enumerate([0, 4, 2, 1, 3]):
                        raw = rawpool.tile([DH, 512], F32R, name="raw", tag="raw")
                        raws[f] = raw
                        if i % 2 == 1:
                            nc.scalar.copy(raw[:], ps[f][:])
                        else:
                            nc.vector.tensor_copy(raw[:], ps[f][:])
                    if sb_i < NQB - 1:
                        for t in range(4):
                            tp = trps.tile([DH, DH], F32R, name="tp", tag="tp")
                            nc.tensor.transpose(
                                tp[:], vt_t[:, t * DH:(t + 1) * DH],
                                identr_sb[:])
                            j = sb_i * 4 + t
                            nc.vector.tensor_copy(vn[:, j * DH:(j + 1) * DH],
                                                  tp[:])
                    return raws, vt_t

                def emit_rope(f, sb_i, raw):
                    # head_dim permuted so pairs sit +-16 apart within each
                    # 32-partition quadrant: dest = p1 + sign*p3 where
                    # p1 = q*cos, p3 = halfswap(q)*sin. dest is bf16.
                    col = slice(sb_i * 512, (sb_i + 1) * 512)
                    dest = qt[f] if f < HPC else kt
                    qs_t = rtp.tile([DH, 512], F32, name="qs_t", tag="qs")
                    p1 = rtp.tile([DH, 512], F32, name="p1", tag="p1")
                    nc.vector.stream_shuffle(qs_t[:], raw[:], _SWAP16)
                    nc.vector.tensor_mul(p1[:], raw[:], csA_sb[:, col])
                    nc.vector.tensor_mul(qs_t[:], qs_t[:], csB_sb[:, col])
                    nc.vector.scalar_tensor_tensor(
                        dest[:, col], qs_t[:], sign_sb[:], p1[:],
                        mybir.AluOpType.mult, mybir.AluOpType.add)

                # PE p-state warmup: ~60 dummy matmuls on garbage SBUF
                # (output never read) ramp the clock to max while the first
                # weight/x DMAs are still in flight
                wz = rawpool.tile([DH, 512], F32, name="wz", tag="raw")
                nc.vector.memset(wz[:, :DH], 0.0)
                for wi in range(15):
                    wps = trps.tile([DH, DH], F32, name="wps", tag="tp")
                    nc.tensor.matmul(wps[:], wz[:, :DH], wz[:, :DH],
                                     start=True, stop=True)

                # ATT0's scores/exp/mask/denominator-accumulate run inside
                # sblock 3's stream (one tiny matmul per d-pair, exp latency
                # fully hidden by the QKV matmuls between), so the ATT0 phase
                # reduces to its PV streams and tails
                a0_es = {}
                a0_accs = []

                def att0_pre(dp):
                    k, h = divmod(dp, HPC)
                    s0 = k * DH
                    e = a0epool.tile([DH, 512], BF16, name="a0e", tag="ae")
                    a0_es[(h, k)] = e
                    sp = trps.tile([DH, 512], F32, name="a0sp", tag="tp")
                    nc.tensor.matmul(sp[:, s0:], kt[:, k * DH:(k + 1) * DH],
                                     qt[h][:, s0:512], start=True, stop=True)
                    nc.scalar.activation(e[:, s0:], sp[:, s0:], EXP)
                    nc.vector.tensor_mul(e[:, s0:s0 + DH], e[:, s0:s0 + DH],
                                         tri_sb[:])
                    if k == 0:
                        acc = a0apool.tile([DH, 512], BF16, name="a0acc",
                                           tag="aa")
                        a0_accs.append(acc)
                        nc.vector.tensor_copy(acc[:], e[:])
                    else:
                        nc.vector.tensor_add(a0_accs[h][:, s0:],
                                             a0_accs[h][:, s0:], e[:, s0:])

                raws3 = vt3 = None
                pending = None
                for sb_i in range(NQB):
                    raws, vt_t = emit_sblock(
                        sb_i, mid=pending,
                        pre=att0_pre if sb_i == NQB - 1 else None)
                    pending = None
                    if sb_i == NQB - 1:
                        # sblock 3's ropes and V transposes are deferred into
                        # phase 2 so their chains don't gate the first scores
                        raws3, vt3 = raws, vt_t
                        continue

                    def pending(raws=raws, sb_i=sb_i):
                        # k first, then q head 0, then the rest
                        for f in [4, 0, 1, 2, 3]:
                            emit_rope(f, sb_i, raws[f])
                if pending is not None:
                    pending()

            # -------- Phase 2: attention (b-outer) interleaved with WO -------
            with (
                tc.tile_pool(name="mps", bufs=8, space="PSUM") as mps,
                tc.tile_pool(name="esb", bufs=10) as epool,
                tc.tile_pool(name="eacc", bufs=4) as eaccp,
                tc.tile_pool(name="bcb", bufs=4) as bcbp,
                tc.tile_pool(name="evsb", bufs=6) as evpool,
            ):
                def wo_units(b, dve_only=True):
                    """The wo projection for q-block b as a list of emission
                    thunks (one PE matmul or one eviction step each), to be
                    drip-fed between attention matmuls of block b+1 so the PE
                    queue never drains on exp latency."""
                    units = []
                    for sti in range(4):
                        st = 4 * b + sti
                        scol = slice(st * DH, (st + 1) * DH)
                        for dh_i in range(2):
                            for jp in range(2):
                                pair = []  # bound late via default args

                                def alloc(pair=pair):
                                    pair.append(mps.tile(
                                        [DH, 512], F32, name="pw", tag="ps"))

                                def mm(jj, h, pair=pair, scol=scol, dh_i=dh_i,
                                       jp=jp):
                                    off = dh_i * 2048 + (jp * 2 + jj) * 512
                                    nc.tensor.matmul(
                                        pair[jj][:], otn[h][:, scol],
                                        wo_sb[:, h * DIM + off:
                                              h * DIM + off + 512],
                                        start=(h == 0), stop=(h == HPC - 1))

                                def ev_emit(pair=pair, scol=scol, dh_i=dh_i,
                                            jp=jp, sti=sti, st=st, b=b,
                                            dve_only=dve_only):
                                    ev = evpool.tile([DH, 1024], F32,
                                                     name="ev", tag="ev")
                                    if dve_only:
                                        # ACT is exp-saturated in the batches
                                        # these units interleave into
                                        nc.vector.tensor_copy(ev[:, 0:512],
                                                              pair[0][:])
                                        nc.vector.tensor_copy(ev[:, 512:1024],
                                                              pair[1][:])
                                    elif (sti + jp) % 2 == 0:
                                        nc.scalar.copy(ev[:, 0:512], pair[0][:])
                                        nc.vector.tensor_copy(ev[:, 512:1024],
                                                              pair[1][:])
                                    else:
                                        nc.vector.tensor_copy(ev[:, 0:512],
                                                              pair[0][:])
                                        nc.scalar.copy(ev[:, 512:1024],
                                                       pair[1][:])
                                    base = dh_i * 2048 + jp * 1024
                                    if b == NQB - 1:
                                        # last batch: halve each out write
                                        # across both rings so the final
                                        # transfer isn't exposed in the tail
                                        nc.sync.dma_start(
                                            out_d[scol, base:base + 512],
                                            ev[:, 0:512])
                                        nc.scalar.dma_start(
                                            out_d[scol, base + 512:base + 1024],
                                            ev[:, 512:1024])
                                    elif (st + dh_i + jp) % 2 == 0:
                                        nc.sync.dma_start(
                                            out_d[scol, base:base + 1024], ev[:])
                                    else:
                                        nc.scalar.dma_start(
                                            out_d[scol, base:base + 1024], ev[:])

                                for jj in range(2):
                                    for h in range(HPC):
                                        if h == 0:
                                            units.append(
                                                (lambda jj=jj, a=alloc, m=mm,
                                                 h=h: (a(), m(jj, h))))
                                        else:
                                            units.append(
                                                (lambda jj=jj, m=mm, h=h:
                                                 m(jj, h)))
                                units.append(ev_emit)
                    return units

                def emit_block(h, b, filler=None):
                    cb = slice(b * 512, (b + 1) * 512)
                    nk = 4 * b + 4  # k tiles contributing to this q block
                    ot_b = mps.tile([DH, 512], F32, name="ot", tag="ps")
                    eacc = eaccp.tile([DH, 512], BF16, name="eacc", tag="ea")
                    e_tiles = [None] * nk

                    def fill():
                        if filler is not None:
                            filler()

                    def s0_of(k):
                        return (k - 4 * b) * DH if k >= 4 * b else 0

                    def emit_scores(k):
                        s0 = s0_of(k)
                        e = epool.tile([DH, 512], BF16, name="E", tag="E")
                        e_tiles[k] = e
                        sp = mps.tile([DH, 512], F32, name="sp", tag="ps")
                        nc.tensor.matmul(
                            sp[:, s0:], kt[:, k * DH:(k + 1) * DH],
                            qt[h][:, b * 512 + s0:(b + 1) * 512],
                            start=True, stop=True)
                        nc.scalar.activation(e[:, s0:], sp[:, s0:], EXP)
                        if k >= 4 * b:
                            # diagonal tile: upper-triangle mask on its strip
                            nc.vector.tensor_mul(
                                e[:, s0:s0 + DH], e[:, s0:s0 + DH], tri_sb[:])
                        if k == 0:
                            nc.vector.tensor_copy(eacc[:], e[:])
                        else:
                            nc.vector.tensor_add(eacc[:, s0:], eacc[:, s0:],
                                                 e[:, s0:])

                    def emit_pv(k):
                        s0 = s0_of(k)
                        nc.tensor.matmul(
                            ot_b[:, s0:], vn[:, k * DH:(k + 1) * DH],
                            e_tiles[k][:, s0:], start=(k == 0),
                            stop=(k == nk - 1), skip_group_check=True)

                    # 3-deep software pipeline: scores run three steps ahead of
                    # PV; wo-projection fill matmuls slot in between
                    depth = min(4, nk)
                    for k in range(depth):
                        emit_scores(k)
                        fill()
                    for k in range(depth, nk):
                        emit_scores(k)
                        fill()
                        emit_pv(k - depth)
                        fill()
                    for k in range(nk - depth, nk):
                        emit_pv(k)
                        fill()

                    # denominator chain, deferred: one ones-matmul reduces the
                    # f32 running sum across partitions (already replicated to
                    # all 128 PSUM partitions, so no broadcast is needed),
                    # recip, then normalize-evict. Returned as a thunk so it
                    # runs a few instructions into the NEXT block, after the
                    # eacc adds have drained.
                    def finish():
                        dn_b = mps.tile([DH, 512], F32, name="dn", tag="ps")
                        nc.tensor.matmul(dn_b[:], ones128_sb[:], eacc[:],
                                         start=True, stop=True)
                        bc = bcbp.tile([DH, 512], F32, name="bc", tag="bc")
                        nc.vector.reciprocal_approx_fast(bc[:], dn_b[:])
                        nc.vector.tensor_mul(otn[h][:, cb], ot_b[:], bc[:])

                    return finish

                def run_batch(b, units):
                    """Emit attention for q-block b, drip-feeding `units`
                    (wo work for block b-1) between its PE instructions."""
                    pend = list(units)
                    state = {"i": 0, "slots": HPC * 2 * (4 * b + 4)}

                    def filler():
                        rem = len(pend) - state["i"]
                        if state["slots"] > 0:
                            n = min(rem, -(-rem // state["slots"]))
                            state["slots"] -= 1
                        else:
                            n = rem
                        for _ in range(n):
                            pend[state["i"]]()
                            state["i"] += 1

                    for h in range(HPC):
                        fin = emit_block(h, b, filler)
                        # the block's denominator/normalize chain drains at
                        # the head of the next block's fill stream, once the
                        # eacc adds have had time to complete
                        pend.insert(min(state["i"] + 2, len(pend)), fin)
                        if b == NQB - 1 and h < HPC - 1:
                            # deferred sblock-3 rope for the next head's
                            # q-block; runs on DVE behind this block's tail
                            emit_rope(h + 1, 3, raws3[h + 1])
                    while state["i"] < len(pend):
                        pend[state["i"]]()
                        state["i"] += 1

                def emit_att0_flat():
                    """b=0's scores/exp ran inside sblock 3; only the PV
                    streams and denominator tails remain here."""
                    nk = 4
                    ots = []
                    for h in range(HPC):
                        ots.append(mps.tile([DH, 512], F32, name=f"ot{h}",
                                            tag="ps"))
                    eaccs = a0_accs

                    for k in range(nk):
                        for h in range(HPC):
                            s0 = k * DH
                            nc.tensor.matmul(ots[h][:, s0:],
                                             vn[:, k * DH:(k + 1) * DH],
                                             a0_es[(h, k)][:, s0:],
                                             start=(k == 0),
                                             stop=(k == nk - 1),
                                             skip_group_check=True)
                    # tails (deferred sblock-3 V transpose + denominator
                    # chain per head) are returned as thunks and drain at the
                    # head of A1's fill stream, under its scores
                    def fin_of(h):
                        def fin():
                            tp = mps.tile([DH, DH], F32R, name="tp", tag="ps")
                            nc.tensor.transpose(
                                tp[:], vt3[:, h * DH:(h + 1) * DH],
                                identr_sb[:])
                            nc.vector.tensor_copy(
                                vn[:, (12 + h) * DH:(13 + h) * DH], tp[:])
                            dn = mps.tile([DH, 512], F32, name="dn", tag="ps")
                            nc.tensor.matmul(dn[:], ones128_sb[:],
                                             eaccs[h][:], start=True,
                                             stop=True)
                            bc = bcbp.tile([DH, 512], F32, name="bc", tag="bc")
                            nc.vector.reciprocal_approx_fast(bc[:], dn[:])
                            nc.vector.tensor_mul(otn[h][:, 0:512], ots[h][:],
                                                 bc[:])
                        return fin

                    return [fin_of(h) for h in range(HPC)]

                a0fins = emit_att0_flat()
                # kt/qt[0] sblock-3 ropes run on DVE during ATT(1)/ATT(2)
                emit_rope(0, 3, raws3[0])
                run_batch(1, a0fins + wo_units(0))
                emit_rope(4, 3, raws3[4])
                run_batch(2, wo_units(1))
                run_batch(3, wo_units(2))
                for u in wo_units(3, dve_only=False):
                    u()

            a0a_ctx.__exit__(None, None, None)
            a0e_ctx.__exit__(None, None, None)
            vt_ctx.__exit__(None, None, None)
            raw_ctx.__exit__(None, None, None)
            rope_ctx.__exit__(None, None, None)

    nc.compile()
    return nc


def get_program():
    global _PROGRAM
    if _PROGRAM is None:
        _PROGRAM = _build_program()
    return _PROGRAM


def make_in_maps(inputs):
    """Host-side sharding / layout prep. Returns one input dict per core."""
    import ml_dtypes
    bf16 = ml_dtypes.bfloat16

    x = np.asarray(inputs["x"], dtype=np.float32)
    wq = np.asarray(inputs["wq"], dtype=np.float32)
    wk = np.asarray(inputs["wk"], dtype=np.float32)
    wv = np.asarray(inputs["wv"], dtype=np.float32)
    wo = np.asarray(inputs["wo"], dtype=np.float32)
    cos = np.asarray(inputs["freqs_cos"], dtype=np.float32)  # (S, 64)
    sin = np.asarray(inputs["freqs_sin"], dtype=np.float32)

    # x transposed then re-laid-out so each [128, 1024] tile (two d-chunks of
    # one 512-wide s-block) is contiguous: [p, pair, sb, c*512+q]
    xT = x.reshape(S, DIM).T  # (DIM, S)
    xT2 = np.ascontiguousarray(
        xT.reshape(NDCH // 2, 2, DH, NQB, 512)
        .transpose(2, 0, 3, 1, 4)
        .reshape(DH, (NDCH // 2) * NQB * 1024)).astype(bf16)

    perm = _head_perm()
    sq = np.float32(DH ** -0.25)  # sqrt of 1/sqrt(head_dim), folded into Q and K
    rows = np.arange(DH)
    pair_idx = 16 * (rows // 32) + (rows % 32) % 16
    csA = np.ascontiguousarray(cos.T[pair_idx] * sq)          # (128, S)
    csB = np.ascontiguousarray(sin.T[pair_idx] * sq)
    sign = np.where((rows % 32) < 16, -1.0, 1.0).astype(np.float32).reshape(DH, 1)
    tri = np.triu(np.ones((DH, DH), dtype=np.float32)).astype(bf16)
    identr = np.eye(DH, dtype=np.float32)
    ones128 = np.ones((DH, DH), dtype=np.float32).astype(bf16)

    wqh = wq.reshape(N_HEADS, DH, DIM)[:, perm, :]
    wkh = wk.reshape(N_KV, DH, DIM)[:, perm, :]
    wvh = wv.reshape(N_KV, DH, DIM)

    in_maps = []
    for c in range(NCORES):
        w_c = np.concatenate(
            [wqh[HPC * c:HPC * (c + 1)].reshape(HPC * DH, DIM),
             wkh[c], wvh[c]], 0)  # (768, DIM)
        wqkvT = np.ascontiguousarray(w_c.T).astype(bf16)  # (DIM, 768)
        woT = np.ascontiguousarray(
            wo[:, HPC * DH * c:HPC * DH * (c + 1)].T).astype(bf16)
        in_maps.append({
            "xT2": xT2, "wqkvT": wqkvT, "woT": woT,
            "csA": csA, "csB": csB, "sign": sign, "tri": tri,
            "identr": identr, "ones128": ones128,
        })
    return in_maps


def _ensure_ntff_hook():
    """The agent image's `antenv` lacks `axon_hooks`; recreate it so
    run_bass_kernel_spmd(trace=True) can capture NTFF profiles. Mirrors
    trn_agent_boot/trn_boot.py::_ntff_profile_via_ctypes."""
    import sys
    try:
        from antenv.axon_hooks import get_axon_ntff_profile_hook  # noqa: F401
        return
    except ImportError:
        pass
    import contextlib
    import ctypes
    import types

    so_path = "/opt/axon/libaxon_pjrt.so"
    hook = None
    try:
        lib = ctypes.CDLL(so_path)
        if hasattr(lib, "axon_start_nrt_profile"):
            lib.axon_start_nrt_profile.argtypes = [
                ctypes.POINTER(ctypes.c_int64), ctypes.c_size_t]
            lib.axon_start_nrt_profile.restype = ctypes.c_int64
            lib.axon_stop_nrt_profile.argtypes = [ctypes.c_char_p]
            lib.axon_stop_nrt_profile.restype = ctypes.c_int64

            @contextlib.contextmanager
            def _hook(output_dir, device_ids):
                import jax
                jax.devices()
                if device_ids:
                    ids = (ctypes.c_int64 * len(device_ids))(*device_ids)
                    rc = lib.axon_start_nrt_profile(ids, len(device_ids))
                else:
                    rc = lib.axon_start_nrt_profile(None, 0)
                if rc != 0:
                    raise RuntimeError(f"axon_start_nrt_profile rc={rc}")
                try:
                    yield
                finally:
                    n = lib.axon_stop_nrt_profile(str(output_dir).encode())
                    print(f"profile: {n} file(s) written to {output_dir}")

            hook = _hook
    except OSError:
        pass

    mod = types.ModuleType("antenv.axon_hooks")
    mod._hook = hook
    mod.get_axon_ntff_profile_hook = lambda: mod._hook
    mod.set_axon_ntff_profile_hook = lambda h: setattr(mod, "_hook", h)
    sys.modules["antenv.axon_hooks"] = mod


def run(inputs, trace=False):
    from concourse.bass_utils import run_bass_kernel_spmd
    if trace:
        _ensure_ntff_hook()
    nc = get_program()
    in_maps = make_in_maps(inputs)
    res = run_bass_kernel_spmd(nc, in_maps, core_ids=list(range(NCORES)),
                               trace=trace)
    acc = np.zeros((S, DIM), dtype=np.float32)
    for r in res.results:
        acc += np.asarray(r["out"], dtype=np.float32)
    return acc.reshape(B, S, DIM), res


def kernel(**inputs):
    out, _ = run(inputs, trace=False)
    return out
